# revision 16
# baseline (speedup 1.0000x reference)
"""Trainium2 Bass kernel for the 4-layer dense transformer (nn_DTransformer).

Self-contained: takes full unsharded inputs, shards across 8 NeuronCores
(sequence-parallel residual stream + head-sharded QK-merge + vocab-sharded
unembed), runs one SPMD Bass/Tile kernel, reassembles the full output.
"""
import sys

sys.path.insert(0, "/opt/trn_rl_repo")

import numpy as np
import ml_dtypes

import concourse.bass as bass
import concourse.mybir as mybir
import concourse.tile as tile
from concourse import bacc
from concourse.bass_utils import run_bass_kernel_spmd
from concourse.masks import make_identity

F32 = mybir.dt.float32
BF16 = mybir.dt.bfloat16
AF = mybir.ActivationFunctionType
ALU = mybir.AluOpType

L, D, H, DV, DM, VOC, NL = 2048, 768, 12, 64, 3072, 32000, 4
NC = 8
R = L // NC            # 256 rows per core
VC = VOC // NC         # 4000 vocab cols per core
ET = D // 128          # 6 feature tiles
JT = DM // 128         # 24 mlp tiles
MT = L // 128          # 16 m (key) tiles
LT = R // 128          # 2 local row tiles
NB = VC // 500         # 8 unembed col blocks
SCALE = float(1.0 / np.sqrt(np.float32(D)))

_CACHE = {}


def _build():
    nc = bacc.Bacc("TRN2", target_bir_lowering=False, debug=False, num_devices=NC)

    # ---------------- I/O ----------------
    e0 = nc.dram_tensor("e0", [R, D], F32, kind="ExternalInput")
    mq = nc.dram_tensor("mq", [NL, 3, D, D], BF16, kind="ExternalInput")
    mk = nc.dram_tensor("mk", [NL, 3, D, 384], BF16, kind="ExternalInput")
    lnp = nc.dram_tensor("lnp", [NL, D, 4], F32, kind="ExternalInput")
    lnf = nc.dram_tensor("lnf", [D, 2], F32, kind="ExternalInput")
    combo = nc.dram_tensor("combo", [NL, D, 87], BF16, kind="ExternalInput")
    woe = nc.dram_tensor("woe", [NL, 76, D], BF16, kind="ExternalInput")
    w1 = nc.dram_tensor("w1", [NL, D, DM], BF16, kind="ExternalInput")
    bm1c = nc.dram_tensor("bm1c", [NL, 128, JT], F32, kind="ExternalInput")
    w2 = nc.dram_tensor("w2", [NL, DM, D], BF16, kind="ExternalInput")
    bm2r = nc.dram_tensor("bm2r", [NL, 1, D], BF16, kind="ExternalInput")
    wue = nc.dram_tensor("wue", [D, VC], BF16, kind="ExternalInput")
    bur = nc.dram_tensor("bur", [1, VC], BF16, kind="ExternalInput")
    out = nc.dram_tensor("out", [L, VC], F32, kind="ExternalOutput")

    # ---------------- internal DRAM ----------------
    xnt_mine = [nc.dram_tensor(f"xnt_mine{i}", [D, R], BF16) for i in range(NL + 1)]
    xnt_all = [
        nc.dram_tensor(f"xnt_all{i}", [NC * D, R], BF16, addr_space="Shared")
        for i in range(NL + 1)
    ]
    mcontrib = [nc.dram_tensor(f"mcon{i}", [3 * D, 384], BF16) for i in range(NL)]
    m_all = [
        nc.dram_tensor(f"mall{i}", [NC * 3 * D, 384], BF16, addr_space="Shared")
        for i in range(NL)
    ]
    esc = nc.dram_tensor("esc", [L, VC], F32)
    denc = nc.dram_tensor("denc", [L], F32)
    den_all = nc.dram_tensor("den_all", [L], F32, addr_space="Shared")

    RG = [list(range(NC))]

    with tile.TileContext(nc) as tc:
        with (
            tc.tile_pool(name="const", bufs=1) as cpool,
            tc.tile_pool(name="pers", bufs=1) as pers,
            tc.tile_pool(name="work", bufs=2) as work,
            tc.tile_pool(name="wt", bufs=1) as wtp,
            tc.tile_pool(name="mqp", bufs=4) as mqp,
            tc.tile_pool(name="mkp", bufs=2) as mkp,
            tc.tile_pool(name="mhp", bufs=8) as mhp,
            tc.tile_pool(name="etp", bufs=3) as etp,
            tc.tile_pool(name="w1p", bufs=6) as w1p,
            tc.tile_pool(name="w2p", bufs=24) as w2p,
            tc.tile_pool(name="gtp", bufs=24) as gtp,
            tc.tile_pool(name="wup", bufs=8) as wup,
            tc.tile_pool(name="eup", bufs=2) as eup,
            tc.tile_pool(name="scp", bufs=2) as scp,
            tc.tile_pool(name="ps", bufs=2, space="PSUM") as ps,
            tc.tile_pool(name="ps3", bufs=2, space="PSUM") as ps3,
        ):
            # constants
            ident = cpool.tile([128, 128], BF16)
            make_identity(nc, ident[:])
            identf = cpool.tile([128, 128], F32)
            make_identity(nc, identf[:])
            ones_row = cpool.tile([1, 256], BF16)   # K=1 matmul lhsT
            nc.vector.memset(ones_row[:], 1.0)

            # residual stream, f32: Y[:, lt*D + d], row l = lt*128 + p
            Y = pers.tile([128, LT * D], F32)
            for lt in range(LT):
                nc.sync.dma_start(Y[:, lt * D:(lt + 1) * D], e0[lt * 128:(lt + 1) * 128, :])

            xnTf = pers.tile([128, ET * L], BF16)   # gathered feature-major LN out

            def layernorm(pcol_g, pcol_b, want_rowmajor):
                """LN of Y chunk -> (lT bf16 [128, ET*R] feature-major, zn f32 or None).

                pcol_g/pcol_b: et -> sbuf [128, 1] f32 gamma/beta column accessors.
                """
                lT = pers.tile([128, ET * R], BF16, tag="lT")
                zn = pers.tile([128, LT * D], F32, tag="zn", name="zn") if want_rowmajor else None
                for lt in range(LT):
                    ys = Y[:, lt * D:(lt + 1) * D]
                    mean = work.tile([128, 1], F32, tag="m1")
                    nc.vector.reduce_sum(mean[:], ys, axis=mybir.AxisListType.X)
                    nmean = work.tile([128, 1], F32, tag="m2")
                    nc.scalar.mul(nmean[:], mean[:], -1.0 / D)
                    cent = work.tile([128, D], F32, tag="cent")
                    nc.vector.tensor_scalar_add(cent[:], ys, nmean[:])
                    sq = work.tile([128, D], F32, tag="sq")
                    ssum = work.tile([128, 1], F32, tag="m3")
                    nc.scalar.activation(sq[:], cent[:], AF.Square, accum_out=ssum[:])
                    var = work.tile([128, 1], F32, tag="m4")
                    nc.scalar.mul(var[:], ssum[:], 1.0 / D)
                    std = work.tile([128, 1], F32, tag="m5")
                    nc.scalar.sqrt(std[:], var[:])
                    rstd = work.tile([128, 1], F32, tag="m6")
                    nc.vector.reciprocal(rstd[:], std[:])
                    norm = work.tile([128, D], BF16, tag="norm")
                    nc.vector.tensor_scalar_mul(norm[:], cent[:], rstd[:])
                    for et in range(ET):
                        pt = ps.tile([128, 128], BF16, tag="tr")
                        nc.tensor.transpose(pt[:], norm[:, et * 128:(et + 1) * 128], ident[:])
                        dst = lT[:, et * R + lt * 128: et * R + (lt + 1) * 128]
                        nc.vector.tensor_scalar(
                            dst, pt[:], pcol_g(et), pcol_b(et),
                            op0=ALU.mult, op1=ALU.add,
                        )
                if want_rowmajor:
                    for lt in range(LT):
                        for et in range(ET):
                            pt = ps.tile([128, 128], BF16, tag="tr")
                            nc.tensor.transpose(
                                pt[:], lT[:, et * R + lt * 128: et * R + (lt + 1) * 128],
                                ident[:],
                            )
                            nc.scalar.copy(zn[:, lt * D + et * 128: lt * D + (et + 1) * 128], pt[:])
                return lT, zn

            def gather_lt(lT, mine_dram, all_dram):
                """DMA local feature-major chunk to DRAM, AllGather, load full."""
                for et in range(ET):
                    nc.sync.dma_start(
                        mine_dram[et * 128:(et + 1) * 128, :],
                        lT[:, et * R:(et + 1) * R],
                    )
                nc.gpsimd.collective_compute(
                    "AllGather", ALU.bypass, replica_groups=RG,
                    ins=[mine_dram[:]], outs=[all_dram[:]],
                )
                # all_dram rows: c*D + et*128 + p, cols l_local -> xnTf[p, et*L + c*R + l]
                v = all_dram[:, :].rearrange("(c e p) l -> e p c l", c=NC, e=ET, p=128)
                for et in range(ET):
                    dst = xnTf[:, et * L:(et + 1) * L].rearrange("p (c l) -> p c l", c=NC)
                    nc.sync.dma_start(dst, v[et])

            # ================= layers =================
            for i in range(NL):
                lnpt = wtp.tile([128, ET * 4], F32, tag="lnp")
                for et in range(ET):
                    nc.sync.dma_start(
                        lnpt[:, et * 4:(et + 1) * 4], lnp[i, et * 128:(et + 1) * 128, :]
                    )
                g1c = lambda et: lnpt[:, et * 4 + 0: et * 4 + 1]
                b1c = lambda et: lnpt[:, et * 4 + 1: et * 4 + 2]
                g2c = lambda et: lnpt[:, et * 4 + 2: et * 4 + 3]
                b2c = lambda et: lnpt[:, et * 4 + 3: et * 4 + 4]

                # ---- M halves (head-sharded QK merge) ----
                for p3 in range(3):
                    mkt = mkp.tile([128, ET * 384], BF16, tag="mk")
                    for kt in range(ET):
                        nc.sync.dma_start(
                            mkt[:, kt * 384:(kt + 1) * 384],
                            mk[i, p3, kt * 128:(kt + 1) * 128, :],
                        )
                    for mt6 in range(ET):
                        mp = ps.tile([128, 384], F32, tag="mm")
                        for kt in range(ET):
                            qt = mqp.tile([128, 128], BF16, tag="mq")
                            nc.sync.dma_start(
                                qt[:],
                                mq[i, p3, kt * 128:(kt + 1) * 128, mt6 * 128:(mt6 + 1) * 128],
                            )
                            nc.tensor.matmul(
                                mp[:], qt[:], mkt[:, kt * 384:(kt + 1) * 384],
                                start=(kt == 0), stop=(kt == ET - 1),
                            )
                        mo = work.tile([128, 384], BF16, tag="mo")
                        nc.scalar.copy(mo[:], mp[:])
                        nc.sync.dma_start(
                            mcontrib[i][p3 * D + mt6 * 128: p3 * D + (mt6 + 1) * 128, :],
                            mo[:],
                        )
                nc.gpsimd.collective_compute(
                    "AllGather", ALU.bypass, replica_groups=RG,
                    ins=[mcontrib[i][:]], outs=[m_all[i][:]],
                )

                # ---- LN1 -> local feature-major + allgather ----
                lT, _ = layernorm(g1c, b1c, want_rowmajor=False)
                gather_lt(lT, xnt_mine[i], xnt_all[i])

                # ---- combo: w (12) | v0 (11) | V11 (64) over all m ----
                cmb = wtp.tile([128, ET * 87], BF16, tag="cmb")
                for et in range(ET):
                    nc.sync.dma_start(
                        cmb[:, et * 87:(et + 1) * 87], combo[i, et * 128:(et + 1) * 128, :]
                    )
                w_sb = pers.tile([128, MT * 12], F32, tag="wsb")
                pvl = pers.tile([128, MT * 76], BF16, tag="pvl")
                for mt in range(MT):
                    cp = ps.tile([128, 87], F32, tag="pv")
                    for et in range(ET):
                        nc.tensor.matmul(
                            cp[:], xnTf[:, et * L + mt * 128: et * L + (mt + 1) * 128],
                            cmb[:, et * 87:(et + 1) * 87],
                            start=(et == 0), stop=(et == ET - 1),
                        )
                    nc.vector.tensor_copy(w_sb[:, mt * 12:(mt + 1) * 12], cp[:, 0:12])
                    nc.vector.tensor_copy(pvl[:, mt * 76: mt * 76 + 75], cp[:, 12:87])
                    nc.vector.memset(pvl[:, mt * 76 + 75: mt * 76 + 76], 1.0)

                # ---- attention heads ----
                ylm = pers.tile([128, LT * 76], BF16, tag="ylm")  # l-major y + ones col
                for lt in range(LT):
                    nc.vector.memset(ylm[:, lt * 76 + 75: lt * 76 + 76], 1.0)
                woet = wtp.tile([76, D], BF16, tag="woe")
                nc.sync.dma_start(woet[:], woe[i])

                for h in range(H):
                    # T^T[h] = M[h](lhsT) @ xnT_local
                    tT = work.tile([128, ET * R], BF16, tag="tT")
                    for n6 in range(ET):
                        g = 2 * h + (n6 // 3)
                        kcore, p3 = g // 3, g % 3
                        c0 = (n6 % 3) * 128
                        tp = ps3.tile([128, R], F32, tag="smm")
                        for et in range(ET):
                            mtile = mhp.tile([128, 128], BF16, tag="mh")
                            nc.sync.dma_start(
                                mtile[:],
                                m_all[i][
                                    kcore * 3 * D + p3 * D + et * 128:
                                    kcore * 3 * D + p3 * D + (et + 1) * 128,
                                    c0:c0 + 128,
                                ],
                            )
                            nc.tensor.matmul(
                                tp[:], mtile[:], lT[:, et * R:(et + 1) * R],
                                start=(et == 0), stop=(et == ET - 1),
                            )
                        nc.scalar.copy(tT[:, n6 * R:(n6 + 1) * R], tp[:])
                    # S^T per m-tile -> exp -> PV accumulate
                    pv = ps.tile([128, R], F32, tag="pv")
                    for mt in range(MT):
                        sp = ps3.tile([128, R], F32, tag="smm")
                        for n6 in range(ET):
                            nc.tensor.matmul(
                                sp[:], xnTf[:, n6 * L + mt * 128: n6 * L + (mt + 1) * 128],
                                tT[:, n6 * R:(n6 + 1) * R],
                                start=(n6 == 0), stop=(n6 == ET - 1),
                            )
                        eT = etp.tile([128, R], BF16, tag="eTm")
                        nc.scalar.activation(
                            eT[:], sp[:], AF.Exp,
                            bias=w_sb[:, mt * 12 + h: mt * 12 + h + 1], scale=SCALE,
                        )
                        nc.tensor.matmul(
                            pv[0:76, :], pvl[:, mt * 76:(mt + 1) * 76],
                            eT[:],
                            start=(mt == 0), stop=(mt == MT - 1),
                        )
                    pv_sb = work.tile([76, R], F32, tag="pvsb")
                    nc.scalar.copy(pv_sb[:], pv[0:76, :])
                    for lt in range(LT):
                        pvT = ps.tile([128, 76], F32, tag="tr")
                        nc.tensor.transpose(
                            pvT[:], pv_sb[:, lt * 128:(lt + 1) * 128], identf[0:76, 0:76]
                        )
                        recip = work.tile([128, 1], F32, tag="recip")
                        nc.vector.reciprocal(recip[:], pvT[:, 75:76])
                        if h < H - 1:
                            nc.vector.tensor_scalar_mul(
                                ylm[:, lt * 76 + h: lt * 76 + h + 1],
                                pvT[:, h:h + 1], recip[:],
                            )
                        else:
                            nc.vector.tensor_scalar_mul(
                                ylm[:, lt * 76 + 11: lt * 76 + 75],
                                pvT[:, 11:75], recip[:],
                            )

                # ---- out-proj + residual: Y = 2Y + yT.T @ [Wo;bo] ----
                yT = pers.tile([76, LT * 128], BF16, tag="yT")
                for lt in range(LT):
                    ytp = ps.tile([128, 128], BF16, tag="tr")
                    nc.tensor.transpose(
                        ytp[0:76, :], ylm[:, lt * 76:(lt + 1) * 76], ident[:]
                    )
                    nc.vector.tensor_copy(yT[:, lt * 128:(lt + 1) * 128], ytp[0:76, :])
                for lt in range(LT):
                    for nb2 in range(2):
                        ap = ps.tile([128, 384], F32, tag="mm")
                        nc.tensor.matmul(
                            ap[:], yT[:, lt * 128:(lt + 1) * 128],
                            woet[:, nb2 * 384:(nb2 + 1) * 384],
                            start=True, stop=True,
                        )
                        ysl = Y[:, lt * D + nb2 * 384: lt * D + (nb2 + 1) * 384]
                        nc.vector.scalar_tensor_tensor(
                            ysl, ysl, 2.0, ap[:], op0=ALU.mult, op1=ALU.add
                        )

                # ---- MLP ----
                znT, zn = layernorm(g2c, b2c, want_rowmajor=True)
                w1t = [w1p.tile([128, DM], BF16, tag="w1", name="w1t") for _ in range(ET)]
                for et in range(ET):
                    nc.sync.dma_start(w1t[et][:], w1[i, et * 128:(et + 1) * 128, :])
                bm1t = wtp.tile([128, JT], F32, tag="bm1")
                nc.sync.dma_start(bm1t[:], bm1c[i])
                gts = []
                for jt in range(JT):
                    hp = ps.tile([128, R], F32, tag="mm")
                    for et in range(ET):
                        nc.tensor.matmul(
                            hp[:], w1t[et][:, jt * 128:(jt + 1) * 128],
                            znT[:, et * R:(et + 1) * R],
                            start=(et == 0), stop=(et == ET - 1),
                        )
                    gt = gtp.tile([128, R], BF16, tag="gT")
                    nc.scalar.activation(
                        gt[:], hp[:], AF.Gelu_apprx_tanh,
                        bias=bm1t[:, jt:jt + 1], scale=1.0,
                    )
                    gts.append(gt)
                w2t = [w2p.tile([128, D], BF16, tag="w2", name="w2t") for _ in range(JT)]
                for jt in range(JT):
                    nc.sync.dma_start(w2t[jt][:], w2[i, jt * 128:(jt + 1) * 128, :])
                bm2t = wtp.tile([1, D], BF16, tag="bm2")
                nc.sync.dma_start(bm2t[:], bm2r[i])
                for lt in range(LT):
                    nc.vector.tensor_add(
                        Y[:, lt * D:(lt + 1) * D], Y[:, lt * D:(lt + 1) * D],
                        zn[:, lt * D:(lt + 1) * D],
                    )
                    for nb2 in range(2):
                        mp2 = ps.tile([128, 384], F32, tag="mm")
                        for jt in range(JT):
                            nc.tensor.matmul(
                                mp2[:], gts[jt][:, lt * 128:(lt + 1) * 128],
                                w2t[jt][:, nb2 * 384:(nb2 + 1) * 384],
                                start=(jt == 0), stop=False,
                            )
                        nc.tensor.matmul(
                            mp2[:], ones_row[:, lt * 128:(lt + 1) * 128],
                            bm2t[:, nb2 * 384:(nb2 + 1) * 384],
                            start=False, stop=True,
                        )
                        ysl = Y[:, lt * D + nb2 * 384: lt * D + (nb2 + 1) * 384]
                        nc.vector.tensor_add(ysl, ysl, mp2[:])

            # ================= final LN + unembed + softmax =================
            lnft = wtp.tile([128, ET * 2], F32, tag="lnf")
            for et in range(ET):
                nc.sync.dma_start(lnft[:, et * 2:(et + 1) * 2], lnf[et * 128:(et + 1) * 128, :])
            gfc = lambda et: lnft[:, et * 2 + 0: et * 2 + 1]
            bfc = lambda et: lnft[:, et * 2 + 1: et * 2 + 2]
            lT, _ = layernorm(gfc, bfc, want_rowmajor=False)
            gather_lt(lT, xnt_mine[NL], xnt_all[NL])

            but = wtp.tile([1, VC], BF16, tag="bu")
            nc.sync.dma_start(but[:], bur[:])
            dens = pers.tile([128, MT * NB], F32, tag="dens")
            for nb in range(NB):
                wut = [wup.tile([128, 500], BF16, tag="wu", name="wut") for _ in range(ET)]
                for et in range(ET):
                    nc.sync.dma_start(
                        wut[et][:], wue[et * 128:(et + 1) * 128, nb * 500:(nb + 1) * 500]
                    )
                for mt in range(MT):
                    up = ps.tile([128, 500], F32, tag="mm")
                    for et in range(ET):
                        nc.tensor.matmul(
                            up[:], xnTf[:, et * L + mt * 128: et * L + (mt + 1) * 128],
                            wut[et][:],
                            start=(et == 0), stop=False,
                        )
                    nc.tensor.matmul(
                        up[:], ones_row[:, 0:128], but[:, nb * 500:(nb + 1) * 500],
                        start=False, stop=True,
                    )
                    eu = eup.tile([128, 500], F32, tag="eu")
                    nc.scalar.activation(
                        eu[:], up[:], AF.Exp,
                        accum_out=dens[:, mt * NB + nb: mt * NB + nb + 1],
                    )
                    nc.sync.dma_start(
                        esc[mt * 128:(mt + 1) * 128, nb * 500:(nb + 1) * 500], eu[:]
                    )
            # local den per l, allreduce, reciprocal, scale pass
            dloc = pers.tile([128, MT], F32, tag="dloc")
            for mt in range(MT):
                nc.vector.reduce_sum(
                    dloc[:, mt:mt + 1], dens[:, mt * NB:(mt + 1) * NB],
                    axis=mybir.AxisListType.X,
                )
                nc.sync.dma_start(denc[mt * 128:(mt + 1) * 128], dloc[:, mt:mt + 1])
            nc.gpsimd.collective_compute(
                "AllReduce", ALU.add, replica_groups=RG,
                ins=[denc[:]], outs=[den_all[:]],
            )
            dall = pers.tile([128, MT], F32, tag="dall")
            nc.sync.dma_start(dall[:], den_all[:].rearrange("(m p) -> p m", p=128))
            drec = pers.tile([128, MT], F32, tag="drec")
            nc.vector.reciprocal(drec[:], dall[:])
            for mt in range(MT):
                for cb in range(8):
                    ei = scp.tile([128, 500], F32, tag="ei")
                    nc.sync.dma_start(
                        ei[:], esc[mt * 128:(mt + 1) * 128, cb * 500:(cb + 1) * 500]
                    )
                    eo = scp.tile([128, 500], F32, tag="eo")
                    nc.vector.tensor_scalar_mul(eo[:], ei[:], drec[:, mt:mt + 1])
                    nc.sync.dma_start(
                        out[mt * 128:(mt + 1) * 128, cb * 500:(cb + 1) * 500], eo[:]
                    )

    nc.compile()
    return nc


def _prep_inputs(inputs):
    bf = ml_dtypes.bfloat16
    x = np.asarray(inputs["x"])
    E0 = (np.asarray(inputs["word_embed"])[x] + np.asarray(inputs["pos_embed"])).astype(np.float32)
    Wq, bq = np.asarray(inputs["Wq"]), np.asarray(inputs["bq"])
    Wk = np.asarray(inputs["Wk"])
    Wv, bv = np.asarray(inputs["Wv"]), np.asarray(inputs["bv"])
    Wo, bo = np.asarray(inputs["Wo"]), np.asarray(inputs["bo"])
    W1, bm1 = np.asarray(inputs["W1"]), np.asarray(inputs["bm1"])
    W2, bm2 = np.asarray(inputs["W2"]), np.asarray(inputs["bm2"])
    Wu, bu = np.asarray(inputs["Wu"]), np.asarray(inputs["bu"])

    lnp = np.stack(
        [np.asarray(inputs["g1"]), np.asarray(inputs["be1"]),
         np.asarray(inputs["g2"]), np.asarray(inputs["be2"])], axis=-1
    ).astype(np.float32)                                   # [NL, D, 4]
    lnf = np.stack([np.asarray(inputs["gf"]), np.asarray(inputs["bef"])], -1).astype(np.float32)

    combo = np.zeros((NL, D, 87), np.float32)
    for i in range(NL):
        for h in range(H):
            combo[i, :, h] = (Wk[i, h] @ bq[i, h]) * SCALE   # u_scaled
        combo[i, :, 12:23] = Wv[i, :11, :, 0].transpose(1, 0)
        combo[i, :, 23:87] = Wv[i, 11]
    woe = np.zeros((NL, 76, D), np.float32)
    for i in range(NL):
        bv_flat = np.concatenate([bv[i, :11, 0], bv[i, 11]])
        woe[i, :75] = Wo[i, :75]
        woe[i, 75] = bo[i] + bv_flat @ Wo[i, :75]
    bm1c = bm1.reshape(NL, JT, 128).transpose(0, 2, 1).astype(np.float32)

    in_maps = []
    for k in range(NC):
        mqk = np.zeros((NL, 3, D, D), np.float32)
        mkk = np.zeros((NL, 3, D, 384), np.float32)
        for p3 in range(3):
            g = 3 * k + p3
            h, s = g // 2, g % 2
            for i in range(NL):
                mqk[i, p3] = Wq[i, h].T
                mkk[i, p3] = Wk[i, h][s * 384:(s + 1) * 384, :].T
        in_maps.append({
            "e0": E0[k * R:(k + 1) * R],
            "mq": mqk.astype(bf),
            "mk": mkk.astype(bf),
            "lnp": lnp,
            "lnf": lnf,
            "combo": combo.astype(bf),
            "woe": woe.astype(bf),
            "w1": W1.astype(bf),
            "bm1c": bm1c,
            "w2": W2.astype(bf),
            "bm2r": bm2.reshape(NL, 1, D).astype(bf),
            "wue": np.ascontiguousarray(Wu[:, k * VC:(k + 1) * VC]).astype(bf),
            "bur": np.ascontiguousarray(bu[None, k * VC:(k + 1) * VC]).astype(bf),
        })
    return in_maps


def _run(inputs, **kw):
    if "nc" not in _CACHE:
        _CACHE["nc"] = _build()
    nc = _CACHE["nc"]
    in_maps = _prep_inputs(inputs)
    res = run_bass_kernel_spmd(nc, in_maps, list(range(NC)), **kw)
    outp = np.concatenate([res.results[k]["out"] for k in range(NC)], axis=1)
    return outp.astype(np.float32), res


def kernel(**inputs):
    outp, _ = _run(inputs)
    return outp


# revision 21
# speedup vs baseline: 1.0865x; 1.0865x over previous
"""Trainium2 Bass kernel for the 4-layer dense transformer (nn_DTransformer).

Self-contained: takes full unsharded inputs, shards across 8 NeuronCores
(sequence-parallel residual stream + head-sharded QK-merge + vocab-sharded
unembed), runs one SPMD Bass/Tile kernel, reassembles the full output.
"""
import sys

sys.path.insert(0, "/opt/trn_rl_repo")

import numpy as np
import ml_dtypes

import concourse.bass as bass
import concourse.mybir as mybir
import concourse.tile as tile
from concourse import bacc
from concourse.bass_utils import run_bass_kernel_spmd
from concourse.masks import make_identity

F32 = mybir.dt.float32
BF16 = mybir.dt.bfloat16
AF = mybir.ActivationFunctionType
ALU = mybir.AluOpType

L, D, H, DV, DM, VOC, NL = 2048, 768, 12, 64, 3072, 32000, 4
NC = 8
R = L // NC            # 256 rows per core
VC = VOC // NC         # 4000 vocab cols per core
ET = D // 128          # 6 feature tiles
JT = DM // 128         # 24 mlp tiles
MT = L // 128          # 16 m (key) tiles
LT = R // 128          # 2 local row tiles
NB = VC // 500         # 8 unembed col blocks
SCALE = float(1.0 / np.sqrt(np.float32(D)))

_CACHE = {}


def _build():
    nc = bacc.Bacc("TRN2", target_bir_lowering=False, debug=False, num_devices=NC)

    # ---------------- I/O ----------------
    e0 = nc.dram_tensor("e0", [R, D], F32, kind="ExternalInput")
    mq = nc.dram_tensor("mq", [NL, 3, D, D], BF16, kind="ExternalInput")
    mk = nc.dram_tensor("mk", [NL, 3, D, 384], BF16, kind="ExternalInput")
    lnp = nc.dram_tensor("lnp", [NL, D, 4], F32, kind="ExternalInput")
    lnf = nc.dram_tensor("lnf", [D, 2], F32, kind="ExternalInput")
    combo = nc.dram_tensor("combo", [NL, D, 87], BF16, kind="ExternalInput")
    woe = nc.dram_tensor("woe", [NL, 76, D], BF16, kind="ExternalInput")
    w1 = nc.dram_tensor("w1", [NL, D, DM], BF16, kind="ExternalInput")
    bm1c = nc.dram_tensor("bm1c", [NL, 128, JT], F32, kind="ExternalInput")
    w2 = nc.dram_tensor("w2", [NL, DM, D], BF16, kind="ExternalInput")
    bm2r = nc.dram_tensor("bm2r", [NL, 1, D], BF16, kind="ExternalInput")
    wue = nc.dram_tensor("wue", [D, VC], BF16, kind="ExternalInput")
    bur = nc.dram_tensor("bur", [1, VC], BF16, kind="ExternalInput")
    out = nc.dram_tensor("out", [L, VC], F32, kind="ExternalOutput")

    # ---------------- internal DRAM ----------------
    xnt_mine = [nc.dram_tensor(f"xnt_mine{i}", [D, R], BF16) for i in range(NL + 1)]
    xnt_all = [
        nc.dram_tensor(f"xnt_all{i}", [NC * D, R], BF16, addr_space="Shared")
        for i in range(NL + 1)
    ]
    mcontrib = [nc.dram_tensor(f"mcon{i}", [3 * D, 384], BF16) for i in range(NL)]
    m_all = [
        nc.dram_tensor(f"mall{i}", [NC * 3 * D, 384], BF16, addr_space="Shared")
        for i in range(NL)
    ]
    esc = nc.dram_tensor("esc", [L, VC], F32)
    denc = nc.dram_tensor("denc", [L], F32)
    den_all = nc.dram_tensor("den_all", [L], F32, addr_space="Shared")

    RG = [list(range(NC))]

    with tile.TileContext(nc) as tc:
        with (
            tc.tile_pool(name="const", bufs=1) as cpool,
            tc.tile_pool(name="pers", bufs=1) as pers,
            tc.tile_pool(name="work", bufs=2) as work,
            tc.tile_pool(name="wt", bufs=1) as wtp,
            tc.tile_pool(name="mqp", bufs=1) as mqp,
            tc.tile_pool(name="mkp", bufs=2) as mkp,
            tc.tile_pool(name="mhp", bufs=3) as mhp,
            tc.tile_pool(name="etp", bufs=3) as etp,
            tc.tile_pool(name="w1p", bufs=6) as w1p,
            tc.tile_pool(name="w2p", bufs=24) as w2p,
            tc.tile_pool(name="gtp", bufs=24) as gtp,
            tc.tile_pool(name="wup", bufs=8) as wup,
            tc.tile_pool(name="eup", bufs=2) as eup,
            tc.tile_pool(name="scp", bufs=3) as scp,
            tc.tile_pool(name="ps", bufs=2, space="PSUM") as ps,
            tc.tile_pool(name="ps3", bufs=2, space="PSUM") as ps3,
        ):
            # constants
            ident = cpool.tile([128, 128], BF16)
            make_identity(nc, ident[:])
            identf = cpool.tile([128, 128], F32)
            make_identity(nc, identf[:])
            ones_row = cpool.tile([1, 256], BF16)   # K=1 matmul lhsT
            nc.vector.memset(ones_row[:], 1.0)

            # residual stream, f32: Y[:, lt*D + d], row l = lt*128 + p
            Y = pers.tile([128, LT * D], F32)
            for lt in range(LT):
                nc.sync.dma_start(Y[:, lt * D:(lt + 1) * D], e0[lt * 128:(lt + 1) * 128, :])

            xnTf = pers.tile([128, ET * L], BF16)   # gathered feature-major LN out

            def layernorm(pcol_g, pcol_b, want_rowmajor):
                """LN of Y chunk -> (lT bf16 [128, ET*R] feature-major, zn f32 or None).

                pcol_g/pcol_b: et -> sbuf [128, 1] f32 gamma/beta column accessors.
                """
                lT = pers.tile([128, ET * R], BF16, tag="lT")
                zn = pers.tile([128, LT * D], BF16, tag="zn", name="zn") if want_rowmajor else None
                for lt in range(LT):
                    ys = Y[:, lt * D:(lt + 1) * D]
                    mean = work.tile([128, 1], F32, tag="m1")
                    nc.vector.reduce_sum(mean[:], ys, axis=mybir.AxisListType.X)
                    nmean = work.tile([128, 1], F32, tag="m2")
                    nc.scalar.mul(nmean[:], mean[:], -1.0 / D)
                    cent = work.tile([128, D], BF16, tag="cent")
                    nc.vector.tensor_scalar_add(cent[:], ys, nmean[:])
                    sq = work.tile([128, D], BF16, tag="sq")
                    ssum = work.tile([128, 1], F32, tag="m3")
                    nc.scalar.activation(sq[:], cent[:], AF.Square, accum_out=ssum[:])
                    var = work.tile([128, 1], F32, tag="m4")
                    nc.scalar.mul(var[:], ssum[:], 1.0 / D)
                    std = work.tile([128, 1], F32, tag="m5")
                    nc.scalar.sqrt(std[:], var[:])
                    rstd = work.tile([128, 1], F32, tag="m6")
                    nc.vector.reciprocal(rstd[:], std[:])
                    norm = work.tile([128, D], BF16, tag="norm")
                    nc.vector.tensor_scalar_mul(norm[:], cent[:], rstd[:])
                    for et in range(ET):
                        pt = ps.tile([128, 128], BF16, tag="tr")
                        nc.tensor.transpose(pt[:], norm[:, et * 128:(et + 1) * 128], ident[:])
                        dst = lT[:, et * R + lt * 128: et * R + (lt + 1) * 128]
                        nc.vector.tensor_scalar(
                            dst, pt[:], pcol_g(et), pcol_b(et),
                            op0=ALU.mult, op1=ALU.add,
                        )
                if want_rowmajor:
                    for lt in range(LT):
                        for et in range(ET):
                            pt = ps.tile([128, 128], BF16, tag="tr")
                            nc.tensor.transpose(
                                pt[:], lT[:, et * R + lt * 128: et * R + (lt + 1) * 128],
                                ident[:],
                            )
                            nc.scalar.copy(zn[:, lt * D + et * 128: lt * D + (et + 1) * 128], pt[:])
                return lT, zn

            def gather_lt(lT, mine_dram, all_dram):
                """DMA local feature-major chunk to DRAM, AllGather, load full."""
                for et in range(ET):
                    nc.sync.dma_start(
                        mine_dram[et * 128:(et + 1) * 128, :],
                        lT[:, et * R:(et + 1) * R],
                    )
                nc.gpsimd.collective_compute(
                    "AllGather", ALU.bypass, replica_groups=RG,
                    ins=[mine_dram[:]], outs=[all_dram[:]],
                )
                # all_dram rows: c*D + et*128 + p, cols l_local -> xnTf[p, et*L + c*R + l]
                v = all_dram[:, :].rearrange("(c e p) l -> e p c l", c=NC, e=ET, p=128)
                for et in range(ET):
                    dst = xnTf[:, et * L:(et + 1) * L].rearrange("p (c l) -> p c l", c=NC)
                    nc.sync.dma_start(dst, v[et])

            # ================= layers =================
            for i in range(NL):
                lnpt = wtp.tile([128, ET * 4], F32, tag="lnp")
                for et in range(ET):
                    nc.sync.dma_start(
                        lnpt[:, et * 4:(et + 1) * 4], lnp[i, et * 128:(et + 1) * 128, :]
                    )
                g1c = lambda et: lnpt[:, et * 4 + 0: et * 4 + 1]
                b1c = lambda et: lnpt[:, et * 4 + 1: et * 4 + 2]
                g2c = lambda et: lnpt[:, et * 4 + 2: et * 4 + 3]
                b2c = lambda et: lnpt[:, et * 4 + 3: et * 4 + 4]

                # ---- M halves (head-sharded QK merge) ----
                for p3 in range(3):
                    mkt = mkp.tile([128, ET * 384], BF16, tag="mk")
                    nc.sync.dma_start(
                        mkt[:].rearrange("p (e c) -> p e c", c=384),
                        mk[i, p3].rearrange("(e p) c -> p e c", p=128),
                    )
                    qt = mqp.tile([128, ET * D], BF16, tag="mq")
                    nc.sync.dma_start(
                        qt[:].rearrange("p (e c) -> p e c", c=D),
                        mq[i, p3].rearrange("(e p) c -> p e c", p=128),
                    )
                    for mt6 in range(ET):
                        mp = ps.tile([128, 384], F32, tag="mm")
                        for kt in range(ET):
                            nc.tensor.matmul(
                                mp[:], qt[:, kt * D + mt6 * 128: kt * D + (mt6 + 1) * 128],
                                mkt[:, kt * 384:(kt + 1) * 384],
                                start=(kt == 0), stop=(kt == ET - 1),
                            )
                        mo = work.tile([128, 384], BF16, tag="mo")
                        nc.scalar.copy(mo[:], mp[:])
                        nc.sync.dma_start(
                            mcontrib[i][p3 * D + mt6 * 128: p3 * D + (mt6 + 1) * 128, :],
                            mo[:],
                        )
                nc.gpsimd.collective_compute(
                    "AllGather", ALU.bypass, replica_groups=RG,
                    ins=[mcontrib[i][:]], outs=[m_all[i][:]],
                )

                # ---- LN1 -> local feature-major + allgather ----
                lT, _ = layernorm(g1c, b1c, want_rowmajor=False)
                gather_lt(lT, xnt_mine[i], xnt_all[i])

                # ---- combo: w (12) | v0 (11) | V11 (64) over all m ----
                cmb = wtp.tile([128, ET * 87], BF16, tag="cmb")
                for et in range(ET):
                    nc.sync.dma_start(
                        cmb[:, et * 87:(et + 1) * 87], combo[i, et * 128:(et + 1) * 128, :]
                    )
                w_sb = pers.tile([128, MT * 12], F32, tag="wsb")
                pvl = pers.tile([128, MT * 76], BF16, tag="pvl")
                for mt in range(MT):
                    cp = ps.tile([128, 87], F32, tag="pv")
                    for et in range(ET):
                        nc.tensor.matmul(
                            cp[:], xnTf[:, et * L + mt * 128: et * L + (mt + 1) * 128],
                            cmb[:, et * 87:(et + 1) * 87],
                            start=(et == 0), stop=(et == ET - 1),
                        )
                    nc.vector.tensor_copy(w_sb[:, mt * 12:(mt + 1) * 12], cp[:, 0:12])
                    nc.vector.tensor_copy(pvl[:, mt * 76: mt * 76 + 75], cp[:, 12:87])
                    nc.vector.memset(pvl[:, mt * 76 + 75: mt * 76 + 76], 1.0)

                # ---- attention heads ----
                ylm = pers.tile([128, LT * 76], BF16, tag="ylm")  # l-major y + ones col
                for lt in range(LT):
                    nc.vector.memset(ylm[:, lt * 76 + 75: lt * 76 + 76], 1.0)
                woet = wtp.tile([76, D], BF16, tag="woe")
                nc.sync.dma_start(woet[:], woe[i])

                for h in range(H):
                    # T^T[h] = M[h](lhsT) @ xnT_local ; load M halves with 2 big DMAs
                    mh_sb = []
                    for s in range(2):
                        g = 2 * h + s
                        kcore, p3 = g // 3, g % 3
                        mhs = mhp.tile([128, ET * 384], BF16, tag="mh", name="mhs")
                        base = kcore * 3 * D + p3 * D
                        nc.sync.dma_start(
                            mhs[:].rearrange("p (e c) -> p e c", c=384),
                            m_all[i][base:base + D, :].rearrange("(e p) c -> p e c", p=128),
                        )
                        mh_sb.append(mhs)
                    tT = work.tile([128, ET * R], BF16, tag="tT")
                    for n6 in range(ET):
                        mhs = mh_sb[n6 // 3]
                        c0 = (n6 % 3) * 128
                        tp = ps3.tile([128, R], F32, tag="smm")
                        for et in range(ET):
                            nc.tensor.matmul(
                                tp[:], mhs[:, et * 384 + c0: et * 384 + c0 + 128],
                                lT[:, et * R:(et + 1) * R],
                                start=(et == 0), stop=(et == ET - 1),
                            )
                        nc.scalar.copy(tT[:, n6 * R:(n6 + 1) * R], tp[:])
                    # S^T per m-tile -> exp -> PV accumulate
                    pv = ps.tile([128, R], F32, tag="pv")
                    for mt in range(MT):
                        sp = ps3.tile([128, R], F32, tag="smm")
                        for n6 in range(ET):
                            nc.tensor.matmul(
                                sp[:], xnTf[:, n6 * L + mt * 128: n6 * L + (mt + 1) * 128],
                                tT[:, n6 * R:(n6 + 1) * R],
                                start=(n6 == 0), stop=(n6 == ET - 1),
                            )
                        eT = etp.tile([128, R], BF16, tag="eTm")
                        nc.scalar.activation(
                            eT[:], sp[:], AF.Exp,
                            bias=w_sb[:, mt * 12 + h: mt * 12 + h + 1], scale=SCALE,
                        )
                        nc.tensor.matmul(
                            pv[0:76, :], pvl[:, mt * 76:(mt + 1) * 76],
                            eT[:],
                            start=(mt == 0), stop=(mt == MT - 1),
                        )
                    pv_sb = work.tile([76, R], F32, tag="pvsb")
                    nc.scalar.copy(pv_sb[:], pv[0:76, :])
                    for lt in range(LT):
                        pvT = ps.tile([128, 76], F32, tag="tr")
                        nc.tensor.transpose(
                            pvT[:], pv_sb[:, lt * 128:(lt + 1) * 128], identf[0:76, 0:76]
                        )
                        recip = work.tile([128, 1], F32, tag="recip")
                        nc.vector.reciprocal(recip[:], pvT[:, 75:76])
                        if h < H - 1:
                            nc.vector.tensor_scalar_mul(
                                ylm[:, lt * 76 + h: lt * 76 + h + 1],
                                pvT[:, h:h + 1], recip[:],
                            )
                        else:
                            nc.vector.tensor_scalar_mul(
                                ylm[:, lt * 76 + 11: lt * 76 + 75],
                                pvT[:, 11:75], recip[:],
                            )

                # ---- out-proj + residual: Y = 2Y + yT.T @ [Wo;bo] ----
                yT = pers.tile([76, LT * 128], BF16, tag="yT")
                for lt in range(LT):
                    ytp = ps.tile([128, 128], BF16, tag="tr")
                    nc.tensor.transpose(
                        ytp[0:76, :], ylm[:, lt * 76:(lt + 1) * 76], ident[:]
                    )
                    nc.vector.tensor_copy(yT[:, lt * 128:(lt + 1) * 128], ytp[0:76, :])
                for lt in range(LT):
                    for nb2 in range(2):
                        ap = ps.tile([128, 384], F32, tag="mm")
                        nc.tensor.matmul(
                            ap[:], yT[:, lt * 128:(lt + 1) * 128],
                            woet[:, nb2 * 384:(nb2 + 1) * 384],
                            start=True, stop=True,
                        )
                        ysl = Y[:, lt * D + nb2 * 384: lt * D + (nb2 + 1) * 384]
                        nc.vector.scalar_tensor_tensor(
                            ysl, ysl, 2.0, ap[:], op0=ALU.mult, op1=ALU.add
                        )

                # ---- MLP ----
                znT, zn = layernorm(g2c, b2c, want_rowmajor=True)
                w1t = [w1p.tile([128, DM], BF16, tag="w1", name="w1t") for _ in range(ET)]
                for et in range(ET):
                    nc.sync.dma_start(w1t[et][:], w1[i, et * 128:(et + 1) * 128, :])
                bm1t = wtp.tile([128, JT], F32, tag="bm1")
                nc.sync.dma_start(bm1t[:], bm1c[i])
                gts = []
                for jt in range(JT):
                    hp = ps.tile([128, R], F32, tag="mm")
                    for et in range(ET):
                        nc.tensor.matmul(
                            hp[:], w1t[et][:, jt * 128:(jt + 1) * 128],
                            znT[:, et * R:(et + 1) * R],
                            start=(et == 0), stop=(et == ET - 1),
                        )
                    gt = gtp.tile([128, R], BF16, tag="gT")
                    nc.scalar.activation(
                        gt[:], hp[:], AF.Gelu_apprx_tanh,
                        bias=bm1t[:, jt:jt + 1], scale=1.0,
                    )
                    gts.append(gt)
                w2t = [w2p.tile([128, D], BF16, tag="w2", name="w2t") for _ in range(JT)]
                for jt in range(JT):
                    nc.sync.dma_start(w2t[jt][:], w2[i, jt * 128:(jt + 1) * 128, :])
                bm2t = wtp.tile([1, D], BF16, tag="bm2")
                nc.sync.dma_start(bm2t[:], bm2r[i])
                for lt in range(LT):
                    nc.vector.tensor_add(
                        Y[:, lt * D:(lt + 1) * D], Y[:, lt * D:(lt + 1) * D],
                        zn[:, lt * D:(lt + 1) * D],
                    )
                    for nb2 in range(2):
                        mp2 = ps.tile([128, 384], F32, tag="mm")
                        for jt in range(JT):
                            nc.tensor.matmul(
                                mp2[:], gts[jt][:, lt * 128:(lt + 1) * 128],
                                w2t[jt][:, nb2 * 384:(nb2 + 1) * 384],
                                start=(jt == 0), stop=False,
                            )
                        nc.tensor.matmul(
                            mp2[:], ones_row[:, lt * 128:(lt + 1) * 128],
                            bm2t[:, nb2 * 384:(nb2 + 1) * 384],
                            start=False, stop=True,
                        )
                        ysl = Y[:, lt * D + nb2 * 384: lt * D + (nb2 + 1) * 384]
                        nc.vector.tensor_add(ysl, ysl, mp2[:])

            # ================= final LN + unembed + softmax =================
            lnft = wtp.tile([128, ET * 2], F32, tag="lnf")
            for et in range(ET):
                nc.sync.dma_start(lnft[:, et * 2:(et + 1) * 2], lnf[et * 128:(et + 1) * 128, :])
            gfc = lambda et: lnft[:, et * 2 + 0: et * 2 + 1]
            bfc = lambda et: lnft[:, et * 2 + 1: et * 2 + 2]
            lT, _ = layernorm(gfc, bfc, want_rowmajor=False)
            gather_lt(lT, xnt_mine[NL], xnt_all[NL])

            but = wtp.tile([1, VC], BF16, tag="bu")
            nc.sync.dma_start(but[:], bur[:])
            dens = pers.tile([128, MT * NB], F32, tag="dens")
            for nb in range(NB):
                wut = [wup.tile([128, 500], BF16, tag="wu", name="wut") for _ in range(ET)]
                for et in range(ET):
                    nc.sync.dma_start(
                        wut[et][:], wue[et * 128:(et + 1) * 128, nb * 500:(nb + 1) * 500]
                    )
                for mt in range(MT):
                    up = ps.tile([128, 500], F32, tag="mm")
                    for et in range(ET):
                        nc.tensor.matmul(
                            up[:], xnTf[:, et * L + mt * 128: et * L + (mt + 1) * 128],
                            wut[et][:],
                            start=(et == 0), stop=False,
                        )
                    nc.tensor.matmul(
                        up[:], ones_row[:, 0:128], but[:, nb * 500:(nb + 1) * 500],
                        start=False, stop=True,
                    )
                    eu = eup.tile([128, 500], F32, tag="eu")
                    nc.scalar.activation(
                        eu[:], up[:], AF.Exp,
                        accum_out=dens[:, mt * NB + nb: mt * NB + nb + 1],
                    )
                    nc.sync.dma_start(
                        esc[mt * 128:(mt + 1) * 128, nb * 500:(nb + 1) * 500], eu[:]
                    )
            # local den per l, allreduce, reciprocal, scale pass
            dloc = pers.tile([128, MT], F32, tag="dloc")
            for mt in range(MT):
                nc.vector.reduce_sum(
                    dloc[:, mt:mt + 1], dens[:, mt * NB:(mt + 1) * NB],
                    axis=mybir.AxisListType.X,
                )
                nc.sync.dma_start(denc[mt * 128:(mt + 1) * 128], dloc[:, mt:mt + 1])
            nc.gpsimd.collective_compute(
                "AllReduce", ALU.add, replica_groups=RG,
                ins=[denc[:]], outs=[den_all[:]],
            )
            dall = pers.tile([128, MT], F32, tag="dall")
            nc.sync.dma_start(dall[:], den_all[:].rearrange("(m p) -> p m", p=128))
            drec = pers.tile([128, MT], F32, tag="drec")
            nc.vector.reciprocal(drec[:], dall[:])
            for mt in range(MT):
                for cb in range(8):
                    ei = scp.tile([128, 500], F32, tag="ei")
                    nc.sync.dma_start(
                        ei[:], esc[mt * 128:(mt + 1) * 128, cb * 500:(cb + 1) * 500]
                    )
                    nc.vector.tensor_scalar_mul(ei[:], ei[:], drec[:, mt:mt + 1])
                    nc.sync.dma_start(
                        out[mt * 128:(mt + 1) * 128, cb * 500:(cb + 1) * 500], ei[:]
                    )

    nc.compile()
    return nc


def _prep_inputs(inputs):
    bf = ml_dtypes.bfloat16
    x = np.asarray(inputs["x"])
    E0 = (np.asarray(inputs["word_embed"])[x] + np.asarray(inputs["pos_embed"])).astype(np.float32)
    Wq, bq = np.asarray(inputs["Wq"]), np.asarray(inputs["bq"])
    Wk = np.asarray(inputs["Wk"])
    Wv, bv = np.asarray(inputs["Wv"]), np.asarray(inputs["bv"])
    Wo, bo = np.asarray(inputs["Wo"]), np.asarray(inputs["bo"])
    W1, bm1 = np.asarray(inputs["W1"]), np.asarray(inputs["bm1"])
    W2, bm2 = np.asarray(inputs["W2"]), np.asarray(inputs["bm2"])
    Wu, bu = np.asarray(inputs["Wu"]), np.asarray(inputs["bu"])

    lnp = np.stack(
        [np.asarray(inputs["g1"]), np.asarray(inputs["be1"]),
         np.asarray(inputs["g2"]), np.asarray(inputs["be2"])], axis=-1
    ).astype(np.float32)                                   # [NL, D, 4]
    lnf = np.stack([np.asarray(inputs["gf"]), np.asarray(inputs["bef"])], -1).astype(np.float32)

    combo = np.zeros((NL, D, 87), np.float32)
    for i in range(NL):
        for h in range(H):
            combo[i, :, h] = (Wk[i, h] @ bq[i, h]) * SCALE   # u_scaled
        combo[i, :, 12:23] = Wv[i, :11, :, 0].transpose(1, 0)
        combo[i, :, 23:87] = Wv[i, 11]
    woe = np.zeros((NL, 76, D), np.float32)
    for i in range(NL):
        bv_flat = np.concatenate([bv[i, :11, 0], bv[i, 11]])
        woe[i, :75] = Wo[i, :75]
        woe[i, 75] = bo[i] + bv_flat @ Wo[i, :75]
    bm1c = bm1.reshape(NL, JT, 128).transpose(0, 2, 1).astype(np.float32)

    in_maps = []
    for k in range(NC):
        mqk = np.zeros((NL, 3, D, D), np.float32)
        mkk = np.zeros((NL, 3, D, 384), np.float32)
        for p3 in range(3):
            g = 3 * k + p3
            h, s = g // 2, g % 2
            for i in range(NL):
                mqk[i, p3] = Wq[i, h].T
                mkk[i, p3] = Wk[i, h][s * 384:(s + 1) * 384, :].T
        in_maps.append({
            "e0": E0[k * R:(k + 1) * R],
            "mq": mqk.astype(bf),
            "mk": mkk.astype(bf),
            "lnp": lnp,
            "lnf": lnf,
            "combo": combo.astype(bf),
            "woe": woe.astype(bf),
            "w1": W1.astype(bf),
            "bm1c": bm1c,
            "w2": W2.astype(bf),
            "bm2r": bm2.reshape(NL, 1, D).astype(bf),
            "wue": np.ascontiguousarray(Wu[:, k * VC:(k + 1) * VC]).astype(bf),
            "bur": np.ascontiguousarray(bu[None, k * VC:(k + 1) * VC]).astype(bf),
        })
    return in_maps


def _run(inputs, **kw):
    if "nc" not in _CACHE:
        _CACHE["nc"] = _build()
    nc = _CACHE["nc"]
    in_maps = _prep_inputs(inputs)
    res = run_bass_kernel_spmd(nc, in_maps, list(range(NC)), **kw)
    outp = np.concatenate([res.results[k]["out"] for k in range(NC)], axis=1)
    return outp.astype(np.float32), res


def kernel(**inputs):
    outp, _ = _run(inputs)
    return outp
